# revision 28
# baseline (speedup 1.0000x reference)
"""Trainium2 Bass kernel for ActorCriticGNN (2-layer GCN + mean-pool + MLP heads).

Strategy (8 NeuronCores, SPMD):
  - Nodes sharded contiguously across cores (NS = N/8 per core); edges
    partitioned by destination shard and sorted by destination tile.
  - Per layer: each core computes its shard of g = deg^-1/2 * (h @ W)
    locally, AllGathers the full bf16 table to HBM, then gathers the
    per-edge source rows with SWDGE dma_gather (int16 indices, table
    split in two halves to fit int16) and scatter-adds them with one-hot
    TensorEngine matmuls into per-destination-tile PSUM accumulators.
  - GCN normalization folded into per-node scales:
      out = deg^-1/2 * (segsum(g[src]) + g) + b,  g = deg^-1/2 * (h @ W)
  - Mean-pool via one-hot matmul over sorted batch ids (+ count column),
    AllReduce of the [97, 64] partial, then tiny data-parallel MLP heads
    computed redundantly on every core.

Host-side work is index manipulation only (graph partitioning / CSR
construction); all floating-point math runs on device.
"""
import math
import numpy as np
import ml_dtypes

import concourse.bass as bass
import concourse.tile as tile
from concourse import bacc, mybir

MSG_BUFS = 12
F32 = mybir.dt.float32
BF16 = mybir.dt.bfloat16
I16 = mybir.dt.int16
I32 = mybir.dt.int32
AF = mybir.ActivationFunctionType
ALU = mybir.AluOpType


class Cfg:
    def __init__(self, N, E, F=96, H=96, G=64, U=32, A=16, n_cores=8,
                 group_tiles=5):
        self.N, self.E, self.F, self.H, self.G, self.U, self.A = N, E, F, H, G, U, A
        self.n_cores = n_cores
        assert N % n_cores == 0
        self.NS = N // n_cores                 # nodes per shard
        self.T = math.ceil(self.NS / 128)      # dst tiles per core
        assert self.NS % 2 == 0
        self.NSH = self.NS // 2                # half-shard rows per core
        self.THALF = self.NSH * n_cores        # rows per half gather table
        assert self.THALF <= 32767             # int16 gather index limit
        self.EPAD = 128                        # padded row width (bf16 -> 256B rows)
        self.GT = min(group_tiles, self.T)     # tiles per gather group


CFG_FULL = Cfg(N=50000, E=800000, group_tiles=2)


def _preprocess(cfg, edge_index, batch):
    """Pure index manipulation: partition edges by destination shard,
    order by (tile, src-half), pad each run to 128-message chunks with a
    structure uniform across cores (SPMD needs one graph)."""
    src = np.asarray(edge_index[0], np.int64)
    dst = np.asarray(edge_index[1], np.int64)
    C, NS, T, NSH = cfg.n_cores, cfg.NS, cfg.T, cfg.NSH
    n_runs = T * 2

    per_core = []
    counts = np.zeros((C, n_runs), np.int64)
    for r in range(C):
        m = (dst >= r * NS) & (dst < (r + 1) * NS)
        es = src[m]
        ed = dst[m] - r * NS
        key = (ed >> 7) * 2 + ((es % NS) >= NSH)
        order = np.argsort(key, kind="stable")
        es, ed, key = es[order], ed[order], key[order]
        counts[r] = np.bincount(key, minlength=n_runs)
        per_core.append((es, ed, key))

    # uniform chunk counts per (tile, half): max over cores
    maxc = counts.max(axis=0)
    Crun = np.ceil(maxc / 128).astype(np.int64)           # [T*2]

    # groups of GT consecutive tiles; global chunk order: grp -> half -> tile
    groups = [list(range(g, min(g + cfg.GT, T))) for g in range(0, T, cfg.GT)]
    runs = []           # (t, h) in global chunk order
    run_gc = {}         # (t,h) -> global chunk offset
    run_lc = {}         # (t,h) -> group-local chunk offset
    run_ni = {}         # (t,h) -> gathered slots (16-aligned, <= chunks*128)
    calls = []          # per group: {h: (slot_off, n_slots, local_chunk_off)}
    gc = 0
    for grp in groups:
        lc = 0
        call = {}
        for h in (0, 1):
            call_c0, call_gc0 = lc, gc
            for t in grp:
                c = int(Crun[t * 2 + h])
                run_gc[(t, h)] = gc
                run_lc[(t, h)] = lc
                gc += c
                lc += c
            n_slots = (lc - call_c0) * 128
            if len(grp) == 1:
                # single-run call: gather only the real indices (16-aligned)
                ni = -(-int(maxc[grp[0] * 2 + h]) // 16) * 16
                run_ni[(grp[0], h)] = ni
            else:
                ni = n_slots
                for t in grp:
                    run_ni[(t, h)] = int(Crun[t * 2 + h]) * 128
            call[h] = (call_gc0 * 128, n_slots, call_c0, ni)
        calls.append(call)
    TOTCH = gc
    CMAX = int(Crun.max()) if len(Crun) else 0
    CGMAX = max(sum(int(Crun[t * 2 + h]) for t in grp for h in (0, 1))
                for grp in groups)

    struct = dict(groups=groups, runs=runs, run_gc=run_gc, run_lc=run_lc,
                  run_ni=run_ni, calls=calls, Crun=Crun, TOTCH=TOTCH,
                  CMAX=CMAX, CGMAX=CGMAX)

    # per-core device arrays
    deg_all = np.bincount(dst, minlength=cfg.N).astype(np.float32)
    batch = np.asarray(batch, np.int64)
    core_data = []
    for r in range(C):
        es, ed, key = per_core[r]
        idx_flat = np.zeros(TOTCH * 128, np.int16)
        dst_flat = np.full(TOTCH * 128, 255.0, np.float32)
        # slice runs out of the sorted arrays
        run_off = np.zeros(n_runs + 1, np.int64)
        np.cumsum(counts[r], out=run_off[1:])
        for t in range(T):
            for h in (0, 1):
                k = t * 2 + h
                a, b = run_off[k], run_off[k + 1]
                n = b - a
                so = run_gc[(t, h)] * 128
                idx_flat[so:so + n] = ((es[a:b] // NS) * NSH
                                       + (es[a:b] % NSH)).astype(np.int16)
                dst_flat[so:so + n] = (ed[a:b] & 127).astype(np.float32)
        # idx wrapped per gather call: [16, len/16] blocks at column so/16
        idx_w16 = np.zeros((16, TOTCH * 8), np.int16)
        for call in calls:
            for h in (0, 1):
                so, ns_, _, ni = call[h]
                if ni == 0:
                    continue
                idx_w16[:, so // 16:(so + ni) // 16] = \
                    idx_flat[so:so + ni].reshape(ni // 16, 16).T
        idx_w = np.tile(idx_w16, (8, 1))                   # replicate per Q7 core
        dl = dst_flat.reshape(TOTCH, 128).T
        dstloc_w = np.repeat(dl, 2, axis=1).astype(ml_dtypes.bfloat16)

        nodes = np.arange(T * 128)
        valid = nodes < NS
        gnodes = np.minimum(nodes, NS - 1) + r * NS
        deg_nm = np.where(valid, deg_all[gnodes], 0.0).astype(np.float32)
        batch_nm = np.where(valid, batch[gnodes].astype(np.float32), 255.0)
        core_data.append(dict(
            idx=idx_w,
            dstloc=dstloc_w,
            deg=deg_nm.reshape(T, 128).T.copy(),           # [128, T]
            batch=batch_nm.reshape(T, 128).T.copy().astype(np.float32),
        ))
    return struct, core_data


def _build(cfg, struct, stage=5):
    N, NS, T, F, H, G, U, A = (cfg.N, cfg.NS, cfg.T, cfg.F, cfg.H, cfg.G,
                               cfg.U, cfg.A)
    EPAD, NSH, THALF = cfg.EPAD, cfg.NSH, cfg.THALF
    TOTCH, CMAX, CGMAX = struct["TOTCH"], struct["CMAX"], struct["CGMAX"]
    groups, calls = struct["groups"], struct["calls"]
    run_gc, run_lc, Crun = struct["run_gc"], struct["run_lc"], struct["Crun"]
    run_ni = struct["run_ni"]
    NTP = T * 128                                          # padded shard nodes
    rg = [list(range(cfg.n_cores))]

    nc = bacc.Bacc("TRN2", target_bir_lowering=False, debug=False,
                   num_devices=cfg.n_cores)
    # ---- I/O ----
    din = {}
    for name, shape, dt in [
        ("xT", [F, NS], F32), ("idx", [128, TOTCH * 8], I16),
        ("dstloc", [128, TOTCH * 2], BF16), ("deg", [128, T], F32),
        ("batch", [128, T], F32), ("uT", [U, G], F32),
        ("W1", [F, H], F32), ("W2", [H, H], F32),
        ("b1", [1, H], F32), ("b2", [1, H], F32),
        ("aW1", [H + U, H], F32), ("aW2", [H, A], F32),
        ("cW1", [H + U, H], F32), ("cW2", [H, 1], F32),
        ("ab1", [H, 1], F32), ("ab2", [A, 1], F32),
        ("cb1", [H, 1], F32), ("cb2", [1, 1], F32),
    ]:
        din[name] = nc.dram_tensor(name, shape, dt, kind="ExternalInput")
    out_logits = nc.dram_tensor("out_logits", [G, A], F32, kind="ExternalOutput")
    out_value = nc.dram_tensor("out_value", [G, 1], F32, kind="ExternalOutput")

    with tile.TileContext(nc) as tc:
        with tc.tile_pool(name="const", bufs=1) as cp, \
             tc.tile_pool(name="big", bufs=1) as bigp, \
             tc.tile_pool(name="msg", bufs=8) as msgp, \
             tc.tile_pool(name="sel", bufs=4) as selp, \
             tc.tile_pool(name="tmp", bufs=4) as tmpp, \
             tc.tile_pool(name="dram", bufs=1, space="DRAM") as dram, \
             tc.tile_pool(name="pscat", bufs=2, space="PSUM") as pscat, \
             tc.tile_pool(name="pl2", bufs=2, space="PSUM") as pl2, \
             tc.tile_pool(name="ptr", bufs=2, space="PSUM") as ptr, \
             tc.tile_pool(name="pmisc", bufs=2, space="PSUM") as pmisc:

            # ---------- constants / persistent state ----------
            W1b = cp.tile([F, H], BF16)
            nc.gpsimd.dma_start(W1b[:], din["W1"][:])
            W2b = cp.tile([H, H], BF16)
            nc.gpsimd.dma_start(W2b[:], din["W2"][:])
            deg = cp.tile([128, T], F32)
            nc.sync.dma_start(deg[:], din["deg"][:])
            bat = cp.tile([128, T], F32)
            nc.sync.dma_start(bat[:], din["batch"][:])
            xT = bigp.tile([F, NTP], BF16)
            nc.gpsimd.memset(xT[:, NS:NTP], 0.0)
            for x0 in range(0, NS, 1280):
                x1 = min(NS, x0 + 1280)
                nc.gpsimd.dma_start(xT[:, x0:x1], din["xT"][:, x0:x1])  # f32->bf16
            idx_sb = bigp.tile([128, TOTCH * 8], I16)
            nc.sync.dma_start(idx_sb[:], din["idx"][:])
            dstloc = bigp.tile([128, max(TOTCH, 1), 2], BF16)
            nc.sync.dma_start(dstloc[:], din["dstloc"][:].rearrange("p (c two) -> p c two", two=2))
            mlp = {}
            for name, shape in [("aW1", [H + U, H]), ("aW2", [H, A]),
                                ("cW1", [H + U, H]), ("cW2", [H, 1]),
                                ("ab1", [H, 1]), ("ab2", [A, 1]),
                                ("cb1", [H, 1]), ("cb2", [1, 1]),
                                ("b1", [1, H]), ("b2", [1, H]),
                                ("uT", [U, G])]:
                mlp[name] = cp.tile(shape, F32, name=f"mlp_{name}", tag=f"mlp_{name}")
                nc.sync.dma_start(mlp[name][:], din[name][:])

            # iota strips
            iota_rep = cp.tile([128, max(CMAX, 1), 64, 2], BF16)
            nc.gpsimd.iota(iota_rep[:], pattern=[[0, max(CMAX, 1)], [2, 64], [1, 2]],
                           base=0, channel_multiplier=0,
                           allow_small_or_imprecise_dtypes=True)
            iotaG = cp.tile([128, G], F32)
            nc.gpsimd.iota(iotaG[:], pattern=[[1, G]], base=0,
                           channel_multiplier=0,
                           allow_small_or_imprecise_dtypes=True)
            # identity (f32) via partition-iota == row-iota
            iota_p = cp.tile([128, 1], I32)
            nc.gpsimd.iota(iota_p[:], pattern=[[0, 1]], base=0,
                           channel_multiplier=1)
            iota_pf = cp.tile([128, 1], F32)
            nc.vector.tensor_copy(iota_pf[:], iota_p[:])
            iota_row = cp.tile([128, 128], F32)
            nc.gpsimd.iota(iota_row[:], pattern=[[1, 128]], base=0,
                           channel_multiplier=0,
                           allow_small_or_imprecise_dtypes=True)
            ident = cp.tile([128, 128], F32)
            nc.vector.tensor_scalar(ident[:], iota_row[:], iota_pf[:], None,
                                    ALU.is_equal)
            ones_row = cp.tile([1, 128], F32)
            nc.gpsimd.memset(ones_row[:], 1.0)

            # dis = rsqrt(deg + 1)
            dtmp = cp.tile([128, T], F32)
            nc.vector.tensor_scalar_add(dtmp[:], deg[:], 1.0)
            drec = cp.tile([128, T], F32)
            nc.vector.reciprocal(drec[:], dtmp[:])
            dis = cp.tile([128, T], F32)
            nc.scalar.sqrt(dis[:], drec[:])

            # B1/B2 bias broadcast tiles via outer product ones x b
            B = {}
            for nm in ("b1", "b2"):
                pb = pmisc.tile([128, H], F32, tag="pp")
                nc.tensor.matmul(pb[:], ones_row[:], mlp[nm][:],
                                 start=True, stop=True)
                B[nm] = cp.tile([128, H], F32, name=f"B{nm}", tag=f"B{nm}")
                nc.vector.tensor_copy(B[nm][:], pb[:])

            # persistent feature state
            g_f32 = bigp.tile([128, T, H], F32)
            h1T = bigp.tile([H, NTP], BF16)
            poolbuf = bigp.tile([128, T, H + 4], BF16)
            nc.gpsimd.memset(poolbuf[:, :, H:H + 1], 1.0)
            pooled = cp.tile([H + 1, G], F32)
            nc.gpsimd.memset(pooled[:], 0.0)

            # DRAM internals (gather tables split in two rank-major halves)
            gtab1 = [dram.tile([THALF, EPAD], BF16, name=f"gtab1{h}",
                               tag=f"gtab1{h}") for h in (0, 1)]
            gtab2 = [dram.tile([THALF, EPAD], BF16, name=f"gtab2{h}",
                               tag=f"gtab2{h}") for h in (0, 1)]
            ag_in1 = [dram.tile([NSH, EPAD], BF16, name=f"agin1{h}",
                                tag=f"agin1{h}") for h in (0, 1)]
            ag_in2 = [dram.tile([NSH, EPAD], BF16, name=f"agin2{h}",
                                tag=f"agin2{h}") for h in (0, 1)]
            ar_in = dram.tile([H + 1, G], F32)
            ar_out = dram.tile([H + 1, G], F32)

            def rows_of(t):
                return min(128, NS - t * 128)

            def shard_write(ag_in, t, gb):
                rw = rows_of(t)
                a_n = min(max(NSH - t * 128, 0), rw)
                if a_n > 0:
                    nc.sync.dma_start(ag_in[0][t * 128:t * 128 + a_n, :],
                                      gb[0:a_n, :])
                if rw - a_n > 0:
                    r0 = t * 128 + a_n - NSH
                    nc.sync.dma_start(ag_in[1][r0:r0 + rw - a_n, :],
                                      gb[a_n:rw, :])

            # ---------- P1: layer-1 local  g1 = dis * (x @ W1) ----------
            for t in range(T):
                ph = pl2.tile([128, H], F32, tag="ph")
                nc.tensor.matmul(ph[:], xT[:, t * 128:(t + 1) * 128], W1b[:],
                                 start=True, stop=True)
                nc.vector.tensor_scalar(g_f32[:, t, :], ph[:], dis[:, t:t + 1],
                                        None, ALU.mult)
                gb = tmpp.tile([128, EPAD], BF16, tag="gb")
                nc.gpsimd.memset(gb[:, H:EPAD], 0.0)
                nc.scalar.copy(gb[:, 0:H], g_f32[:, t, :])
                shard_write(ag_in1, t, gb)

            def dummy_out():
                zl = cp.tile([G, A], F32, name="zl", tag="zl")
                nc.gpsimd.memset(zl[:], 0.0)
                nc.sync.dma_start(out_logits[:], zl[:, 0:A])
                nc.sync.dma_start(out_value[:], zl[:, 0:1])
            scratch = dram.tile([128, T, H], F32)

            # ---------- AG1 (two half-table collectives) ----------
            def ag_emit(ag_in, gtab, h):
                if cfg.n_cores > 1:
                    nc.gpsimd.collective_compute(
                        "AllGather", ALU.bypass, replica_groups=rg,
                        ins=[ag_in[h][:].opt()], outs=[gtab[h][:].opt()])
                else:
                    nc.sync.dma_start(gtab[h][:], ag_in[h][:])

            ag_emit(ag_in1, gtab1, 0)

            LAGK = 8

            def scatter_layer(lid, gtab, epilogue, mid=None):
                """Lagged two-stream gather + one-hot scatter.

                A-half gathers run LAGK tiles ahead so the DMA engines have
                issued work while the B-half table's AllGather is still in
                flight (the Pool queue blocks in-order on that wait).
                Requires GT=1 (one tile per call group).
                """
                mt = {}

                def emit_gather(t, h):
                    so, ns_, lc0r, ni = calls[t][h]
                    if ni == 0:
                        mt[(t, h)] = None
                        return
                    nchk = -(-ni // 128)
                    pool_h = msgpA if h == 0 else msgpB
                    m = pool_h.tile([128, max(CMAX, 1), EPAD], BF16,
                                    tag=f"m{h}", name=f"msg{lid}_{t}_{h}")
                    nc.gpsimd.dma_gather(
                        m[:, 0:nchk, :], gtab[h][:],
                        idx_sb[:, so // 16:(so + ni) // 16],
                        num_idxs=ni, num_idxs_reg=ni, elem_size=EPAD,
                        single_packet=False)
                    mt[(t, h)] = m

                def process(t):
                    acc = pscat.tile([128, H], F32, tag="acc")
                    hs = [h for h in (0, 1) if Crun[t * 2 + h] > 0]
                    if not hs:
                        nc.vector.memset(acc[:], 0.0)
                    first = True
                    for h in hs:
                        cth = int(Crun[t * 2 + h])
                        gc0 = run_gc[(t, h)]
                        msg = mt.pop((t, h))
                        selt = selp.tile([128, max(CMAX, 1), 64, 2], BF16,
                                         tag="sel")
                        nc.vector.tensor_tensor(
                            selt[:, 0:cth, :, :], iota_rep[:, 0:cth, :, :],
                            dstloc[:, gc0:gc0 + cth, :].unsqueeze(2)
                                  .to_broadcast((128, cth, 64, 2)),
                            ALU.is_equal)
                        rni = run_ni[(t, h)]
                        for c in range(cth):
                            kc = min(128, rni - c * 128)
                            nc.tensor.matmul(
                                acc[:], selt[0:kc, c, :, :].opt(),
                                msg[0:kc, c, 0:H],
                                start=first,
                                stop=(h == hs[-1] and c == cth - 1))
                            first = False
                    epilogue(t, acc)

                for t in range(min(LAGK, T)):
                    emit_gather(t, 0)
                if mid is not None:
                    mid()
                for t in range(T):
                    emit_gather(t, 1)
                    if t + LAGK < T:
                        emit_gather(t + LAGK, 0)
                    process(t)

            # ---------- P3: layer-1 scatter -> h1 -> layer-2 local ----------
            def epi1(t, acc):
                e1 = tmpp.tile([128, H], F32, tag="e1")
                nc.vector.tensor_tensor(e1[:], acc[:], g_f32[:, t, :], ALU.add)
                e2 = tmpp.tile([128, H], F32, tag="e2")
                nc.scalar.activation(e2[:], e1[:], AF.Copy, bias=0.0,
                                     scale=dis[:, t:t + 1])
                e3 = tmpp.tile([128, H], F32, tag="e3")
                nc.vector.tensor_tensor(e3[:], e2[:], B["b1"][:], ALU.add)
                h1 = tmpp.tile([128, H], F32, tag="h1")
                nc.vector.tensor_scalar_max(h1[:], e3[:], 0.0)
                # transpose -> h1T (bf16)
                trp = ptr.tile([H, 128], F32, tag="trp")
                nc.tensor.transpose(trp[:], h1[:], ident[:])
                nc.scalar.copy(h1T[:, t * 128:(t + 1) * 128], trp[:])
                # layer-2 local matmul + g2
                ph2 = pl2.tile([128, H], F32, tag="ph")
                nc.tensor.matmul(ph2[:], h1T[:, t * 128:(t + 1) * 128], W2b[:],
                                 start=True, stop=True)
                nc.vector.tensor_scalar(g_f32[:, t, :], ph2[:],
                                        dis[:, t:t + 1], None, ALU.mult)
                gb = tmpp.tile([128, EPAD], BF16, tag="gb")
                nc.gpsimd.memset(gb[:, H:EPAD], 0.0)
                nc.scalar.copy(gb[:, 0:H], g_f32[:, t, :])
                shard_write(ag_in2, t, gb)

            done = False
            if stage == 1:
                nc.sync.dma_start(scratch[:, 0, :], g_f32[:, 0, :])
                dummy_out()
                done = True
            elif stage == 2:
                def epi_dbg(t, acc):
                    e1 = tmpp.tile([128, H], F32, tag="e1")
                    nc.vector.tensor_tensor(e1[:], acc[:], g_f32[:, t, :], ALU.add)
                    nc.sync.dma_start(scratch[:, t, :], e1[:])
                scatter_layer(0, gtab1, epi_dbg, mid=lambda: ag_emit(ag_in1, gtab1, 1))
                dummy_out()
                done = True
            else:
                scatter_layer(1, gtab1, epi1, mid=lambda: ag_emit(ag_in1, gtab1, 1))

            # ---------- AG2 (two half-table collectives) ----------
            if not done:
                ag_emit(ag_in2, gtab2, 0)

            # ---------- P5: layer-2 scatter -> h2 -> pooling ----------
            def epi2(t, acc):
                e1 = tmpp.tile([128, H], F32, tag="e1")
                nc.vector.tensor_tensor(e1[:], acc[:], g_f32[:, t, :], ALU.add)
                e2 = tmpp.tile([128, H], F32, tag="e2")
                nc.scalar.activation(e2[:], e1[:], AF.Copy, bias=0.0,
                                     scale=dis[:, t:t + 1])
                e3 = tmpp.tile([128, H], F32, tag="e3")
                nc.vector.tensor_tensor(e3[:], e2[:], B["b2"][:], ALU.add)
                nc.vector.tensor_scalar_max(poolbuf[:, t, 0:H], e3[:], 0.0)
                selg = tmpp.tile([128, G], BF16, tag="selg")
                nc.vector.tensor_scalar(selg[:], iotaG[:], bat[:, t:t + 1],
                                        None, ALU.is_equal)
                pp = pmisc.tile([H + 1, G], F32, tag="pp")
                nc.tensor.matmul(pp[:], poolbuf[:, t, 0:H + 1], selg[:],
                                 start=True, stop=True)
                nc.vector.tensor_tensor(pooled[:], pooled[:], pp[:], ALU.add)

            if not done and stage == 3:
                dummy_out()
                done = True
            if not done:
                scatter_layer(2, gtab2, epi2, mid=lambda: ag_emit(ag_in2, gtab2, 1))
            if not done and stage == 4:
                nc.sync.dma_start(scratch[:, 0, 0:H], poolbuf[:, 0, 0:H])
                dummy_out()
                done = True

            # ---------- AR pooled ----------
            if done:
                pAR = None
            else:
                nc.sync.dma_start(ar_in[:], pooled[:])
                if cfg.n_cores > 1:
                    nc.gpsimd.collective_compute(
                        "AllReduce", ALU.add, replica_groups=rg,
                        ins=[ar_in[:].opt()], outs=[ar_out[:].opt()])
                else:
                    nc.sync.dma_start(ar_out[:], ar_in[:])
                pAR = cp.tile([H + 1, G], F32)
                nc.sync.dma_start(pAR[:], ar_out[:])

            # ---------- MLP heads (redundant on every core) ----------
            if done:
                _skip_mlp(nc)
                cnt = None
            if not done:
                cnt = cp.tile([1, G], F32)
            nc.vector.tensor_scalar_max(cnt[:], pAR[H:H + 1, :], 1.0)
            rec = cp.tile([1, G], F32)
            nc.vector.reciprocal(rec[:], cnt[:])
            prb = pmisc.tile([H, G], F32, tag="pp")
            nc.tensor.matmul(prb[:], ones_row[:, 0:H], rec[:],
                             start=True, stop=True)
            combT = cp.tile([H + U, G], F32)
            nc.vector.tensor_tensor(combT[0:H, :], pAR[0:H, :], prb[:],
                                    ALU.mult)
            nc.sync.dma_start(combT[H:H + U, :], din["uT"][:])

            def head(Wn1, bn1, Wn2, bn2, odim, out_ext):
                p1 = pmisc.tile([H, G], F32, tag="pp")
                nc.tensor.matmul(p1[:], mlp[Wn1][:], combT[:],
                                 start=True, stop=True)
                z = cp.tile([H, G], F32, name=f"z{Wn1}", tag=f"z{Wn1}")
                nc.scalar.activation(z[:], p1[:], AF.Relu, bias=mlp[bn1][:],
                                     scale=1.0)
                p2 = pmisc.tile([A, G], F32, tag="pp")
                nc.tensor.matmul(p2[0:odim, :], mlp[Wn2][:], z[:],
                                 start=True, stop=True)
                oT = cp.tile([A, G], F32, name=f"oT{Wn1}", tag=f"oT{Wn1}")
                nc.scalar.activation(oT[0:odim, :], p2[0:odim, :], AF.Identity,
                                     bias=mlp[bn2][:], scale=1.0)
                pL = ptr.tile([G, A], F32, tag="trp")
                nc.tensor.matmul(pL[:, 0:odim], oT[0:odim, :],
                                 ident[0:odim, 0:odim], start=True, stop=True)
                ores = cp.tile([G, A], F32, name=f"or{Wn1}", tag=f"or{Wn1}")
                nc.vector.tensor_copy(ores[:, 0:odim], pL[:, 0:odim])
                nc.sync.dma_start(out_ext[:], ores[:, 0:odim])

            head("aW1", "ab1", "aW2", "ab2", A, out_logits)
            head("cW1", "cb1", "cW2", "cb2", 1, out_value)

    nc.compile()
    return nc


def _finish(nc):
    return None


def _in_maps(cfg, inputs, core_data):
    x = np.asarray(inputs["x"], np.float32)
    maps = []
    for r in range(cfg.n_cores):
        cd = core_data[r]
        sh = slice(r * cfg.NS, (r + 1) * cfg.NS)
        maps.append({
            "xT": np.ascontiguousarray(x[sh].T),
            "idx": cd["idx"], "dstloc": cd["dstloc"],
            "deg": cd["deg"], "batch": cd["batch"],
            "uT": np.ascontiguousarray(np.asarray(inputs["u"], np.float32).T),
            "W1": np.asarray(inputs["W1"], np.float32),
            "W2": np.asarray(inputs["W2"], np.float32),
            "b1": np.asarray(inputs["b1"], np.float32).reshape(1, -1),
            "b2": np.asarray(inputs["b2"], np.float32).reshape(1, -1),
            "aW1": np.asarray(inputs["aW1"], np.float32),
            "aW2": np.asarray(inputs["aW2"], np.float32),
            "cW1": np.asarray(inputs["cW1"], np.float32),
            "cW2": np.asarray(inputs["cW2"], np.float32),
            "ab1": np.asarray(inputs["ab1"], np.float32).reshape(-1, 1),
            "ab2": np.asarray(inputs["ab2"], np.float32).reshape(-1, 1),
            "cb1": np.asarray(inputs["cb1"], np.float32).reshape(-1, 1),
            "cb2": np.asarray(inputs["cb2"], np.float32).reshape(-1, 1),
        })
    return maps


def build_all(cfg, inputs):
    struct, core_data = _preprocess(cfg, inputs["edge_index"], inputs["batch"])
    nc = _build(cfg, struct)
    return nc, _in_maps(cfg, inputs, core_data)


def kernel(**inputs):
    from concourse.bass_utils import run_bass_kernel_spmd
    cfg = CFG_FULL
    nc, in_maps = build_all(cfg, inputs)
    res = run_bass_kernel_spmd(nc, in_maps, core_ids=list(range(cfg.n_cores)))
    logits = np.asarray(res.results[0]["out_logits"], np.float32)
    value = np.asarray(res.results[0]["out_value"], np.float32)
    return logits, value
